# revision 8
# baseline (speedup 1.0000x reference)
"""Trainium2 Bass kernel for char-CNN (embed lookup + conv1d(K=5,pad=2) + bias + maxpool).

Math: out[n, f] = max_w ( b[f] + sum_k sum_d  E[ids[n, w+k-2], d] * Wc[f, d, k] )

v2 strategy (data-parallel over 8 cores, 4096 tokens each):
  * Host folds embedding+conv:  G[k][v, f] = sum_d E[v, d] * Wc[f, d, k],
    split hi/lo fp16 (hi+lo carries ~22 mantissa bits).
  * Table lookups run on the TensorEngine as one-hot matmuls over vocab 96
    (+1 ones row carrying the bias on the center tap). 10 matmuls per
    32-token unit: 2 splits x 5 taps, PSUM-accumulated, N=512 cols each.
  * One-hot layout is w-major [vocab, W+4, 32 tokens] so every tap's moving
    operand is FULLY CONTIGUOUS: strided moving costs ~8% on the PE stream
    (233 vs 216 ns measured), w-major removes it. ids are host-transposed.
  * The one-hot is written ONCE per unit in fp8-e4m3 (exact for 0/1): DVE
    is_equal with fp8 output measured 379ns vs 1020ns for fp16 output, and
    fp16-weight x fp8-moving matmuls are exact (verified on HW).
  * Pad columns compare against -1 so the full-window is_equal writes exact
    zeros every unit; no separate memset of an fp8 tile needed.
  * PSUM pairs (2 banks) let one fused reduce_max cover 64 tokens; 4
    pair-buffers = all 8 banks keep the in-order PE queue saturated.
  * Output [group, F, 512] per core; host transposes/concats.
"""

import numpy as np

import concourse.bass as bass
import concourse.bacc as bacc
import concourse.mybir as mybir
from concourse.tile import TileContext
from concourse.bass_utils import run_bass_kernel_spmd

N, W = 32768, 16
VOCAB, D, F, K = 96, 100, 100, 5
N_CORES = 8
NSH = N // N_CORES            # 4096 tokens per core
UNIT = 32                     # tokens per unit (512 one-hot cols per tap)
NUNIT = NSH // UNIT           # 128
SUPER = 4                     # units per superblock (= 2 PSUM pairs)
NSUPER = NUNIT // SUPER       # 32
GROUP = 512                   # tokens per ids DMA / out DMA
NGROUP = NSH // GROUP         # 8
UPG = GROUP // UNIT           # 16 units per group
SPG = GROUP // (SUPER * UNIT)  # 4 superblocks per group
VP = VOCAB + 1                # + ones row (bias)
WP = W + 4                    # padded char positions
FP = 128                      # F padded to 128 (enables FWL)
OHB = 10                      # one-hot buffers in rotation
BCZB = 4                      # broadcast buffers

bf16 = mybir.dt.bfloat16
f16 = mybir.dt.float16
f32 = mybir.dt.float32
f8e4 = mybir.dt.float8e4
i32 = mybir.dt.int32


def build_nc():
    nc = bacc.Bacc("TRN2", target_bir_lowering=False)

    ids_d = nc.dram_tensor("ids", [NUNIT, W, UNIT], bf16, kind="ExternalInput")
    # hi tables at [:, 0:K, :], lo tables at [:, K:2K, :]
    g16_d = nc.dram_tensor("g16", [VP, 2 * K, FP], f16, kind="ExternalInput")
    iota_d = nc.dram_tensor("iota", [VOCAB, 1], f32, kind="ExternalInput")
    out_d = nc.dram_tensor("out", [NGROUP, F, GROUP], f32, kind="ExternalOutput")

    with TileContext(nc) as tc:
        with (
            tc.tile_pool(name="consts", bufs=1) as consts,
            tc.tile_pool(name="outp", bufs=2) as outp,
            tc.tile_pool(name="idsp", bufs=6) as idsp,
            tc.tile_pool(name="psp", bufs=4, space="PSUM") as psp,
        ):
            ids_tiles = {}
            QU = 4  # units per ids load (small first DMAs unblock builds fast)

            def load_ids(q):
                idst = idsp.tile([1, QU * UNIT * W], bf16, tag="ids")
                nc.scalar.dma_start(
                    out=idst[0:1, :],
                    in_=ids_d[q * QU : (q + 1) * QU, :, :].rearrange(
                        "u w t -> (u w t)"
                    ).unsqueeze(0),
                )
                ids_tiles[q] = idst

            iota_t = consts.tile([VOCAB, 1], f32)
            nc.sync.dma_start(out=iota_t, in_=iota_d[:, :])
            load_ids(0)
            load_ids(1)
            load_ids(2)
            # DVE warm-up: absorb first-dispatch latency during init
            dve_warm = consts.tile([VOCAB, 2], f32, tag="dve_warm")
            nc.vector.tensor_scalar(
                out=dve_warm[:, 0:1],
                in0=iota_t[:, :],
                scalar1=iota_t[:, 0:1],
                scalar2=None,
                op0=mybir.AluOpType.is_equal,
            )
            nc.vector.reduce_max(
                out=dve_warm[:, 1:2],
                in_=iota_t[:, :],
                axis=mybir.AxisListType.X,
            )

            g16t = consts.tile([VP, 2 * K, FP], f16)
            nc.sync.dma_start(
                out=g16t.rearrange("v k f -> v (k f)"),
                in_=g16_d.rearrange("v k f -> v (k f)"),
            )

            # broadcast + one-hot buffers
            bcz_tiles = []
            for j in range(BCZB):
                bt = consts.tile([VOCAB, WP, UNIT], bf16, tag=f"bcz{j}")
                bcz_tiles.append(bt)
            nc.vector.memset(bcz_tiles[0][:, :, :], -1.0)
            oh_tiles = []
            for j in range(OHB):
                o8 = consts.tile([VP, WP, UNIT], f8e4, tag=f"oh8_{j}")
                oh_tiles.append(o8)

            # scratch consts: ones source (bias rows / gpsimd warm) and a
            # ones column for the PE-broadcast matmuls
            warmb = consts.tile([FP, WP * UNIT], bf16, tag="warmb")
            nc.vector.memset(warmb[:, :], 1.0)
            onescol = consts.tile([1, FP], bf16, tag="onescol")
            nc.vector.memset(onescol[:, :], 1.0)

            def fill_ones(j):
                # bias row: constant 1.0, written by the otherwise-idle ACT
                nc.scalar.copy(
                    out=oh_tiles[j][VOCAB : VOCAB + 1, :, :],
                    in_=warmb[VOCAB : VOCAB + 1, :],
                )

            def fill_pads(j):
                # both pad-column blocks ({0,1} and {18,19}) in one DVE op:
                # is_equal(-1 == iota) writes exact zeros
                nc.vector.tensor_scalar(
                    out=oh_tiles[j][0:VOCAB, :, :].rearrange(
                        "v (b w) t -> v b w t", b=WP // 2
                    )[:, 0 :: (WP // 2 - 1), 0:2, :],
                    in0=bcz_tiles[0][:, :, :].rearrange(
                        "v (b w) t -> v b w t", b=WP // 2
                    )[:, 0 :: (WP // 2 - 1), 0:2, :],
                    scalar1=iota_t[:, 0:1],
                    scalar2=None,
                    op0=mybir.AluOpType.is_equal,
                )

            for j in range(SUPER):
                fill_ones(j)
                fill_pads(j)

            # GpSimd Q7 warm-up: the first partition_broadcast can pay ~12us
            # of cold ucode load; run a tiny one now so it overlaps.
            nc.gpsimd.partition_broadcast(
                out_ap=warmb[0:1, 2:4],
                in_ap=warmb[0:1, 0:2],
                channels=1,
            )

            def build(u):
                q, uu = divmod(u, QU)
                bcz = bcz_tiles[u % BCZB]
                idst = ids_tiles[q]
                nc.gpsimd.partition_broadcast(
                    out_ap=bcz[:, 2 : 2 + W, :],
                    in_ap=idst[0:1, uu * (W * UNIT) : (uu + 1) * (W * UNIT)],
                    channels=VOCAB,
                )
                nc.vector.tensor_scalar(
                    out=oh_tiles[u % OHB][0:VOCAB, :, :],
                    in0=bcz[:, :, :],
                    scalar1=iota_t[:, 0:1],
                    scalar2=None,
                    op0=mybir.AluOpType.is_equal,
                )

            def psum_build(u, ps_slot):
                # PE-side ids broadcast: ones[1,96]^T @ ids[1,512] -> PSUM,
                # then one-hot via is_equal straight from PSUM. Lets the
                # first 3 superblocks start without GpSimd or replication
                # DMAs; also rides the PE p-state ramp.
                q, uu = divmod(u, QU)
                idst = ids_tiles[q]
                nc.tensor.matmul(
                    ps_slot[0:VOCAB, :, :],
                    onescol[0:1, 0:VOCAB],
                    idst[0:1, uu * (W * UNIT) : (uu + 1) * (W * UNIT)],
                    start=True,
                    stop=True,
                    skip_group_check=True,
                )
                nc.vector.tensor_scalar(
                    out=oh_tiles[u % OHB][0:VOCAB, 2 : 2 + W, :],
                    in0=ps_slot[0:VOCAB, :, :],
                    scalar1=iota_t[:, 0:1],
                    scalar2=None,
                    op0=mybir.AluOpType.is_equal,
                )

            out_sb = None
            for S in range(NSUPER):
                g, ss = divmod(S, SPG)
                if ss == 0:
                    out_sb = outp.tile([FP, GROUP], f32, tag="osb")
                if S + 3 < NSUPER:
                    load_ids(S + 3)

                units = [SUPER * S + j for j in range(SUPER)]
                pairs = [
                    psp.tile([FP, 2, W, UNIT], f32, tag="pair", name=f"pr{S}_{p}")
                    for p in range(2)
                ]
                if S < 2:
                    # bootstrap: broadcast this superblock's ids through its
                    # own PSUM pairs before the accumulation resets them
                    for j, u in enumerate(units):
                        psum_build(u, pairs[j // 2][:, j % 2, :, :])
                if S == 0:
                    # deferred init, behind the bootstrap on the DVE queue
                    for j in range(SUPER, OHB):
                        fill_ones(j)
                        fill_pads(j)
                    for j in range(1, BCZB):
                        nc.vector.memset(bcz_tiles[j][:, :, :], -1.0)
                # 2 splits x 5 taps, k-outer for weight reuse across 4 units
                for s in range(2):
                    for k in range(K):
                        for j, u in enumerate(units):
                            nc.tensor.matmul(
                                pairs[j // 2][:, j % 2, :, :],
                                g16t[:, s * K + k, :],
                                oh_tiles[u % OHB][:, k : k + W, :],
                                start=(s == 0 and k == 0),
                                stop=(s == 1 and k == K - 1),
                                skip_group_check=True,
                            )

                # build one-hots two superblocks ahead (gpsimd path)
                for j in range(SUPER):
                    u = SUPER * (S + 2) + j
                    if 2 * SUPER <= u < NUNIT:
                        build(u)

                # fused reduce over each PSUM pair (2 banks, 64 tokens)
                for p in range(2):
                    cols = ss * (SUPER * UNIT) + p * (2 * UNIT)
                    nc.vector.reduce_max(
                        out=out_sb[0:F, cols : cols + 2 * UNIT].rearrange(
                            "f (p t) -> f p t", p=2
                        ),
                        in_=pairs[p][0:F, :, :, :].rearrange("f p w t -> f p t w"),
                        axis=mybir.AxisListType.X,
                    )

                if ss == SPG - 1:
                    nc.sync.dma_start(
                        out=out_d[g, :, :],
                        in_=out_sb[0:F, :],
                    )

    nc.compile()
    return nc


def make_consts(embed_table, conv_w, conv_b):
    G = np.einsum(
        "vd,fdk->kvf", embed_table.astype(np.float64), conv_w.astype(np.float64)
    )
    Gf = np.zeros((K, VP, F), np.float64)
    Gf[:, 0:VOCAB, :] = G
    Gf[2, VOCAB, :] = conv_b.astype(np.float64)  # bias rides center tap

    hi = Gf.astype(np.float32).astype(np.float16)
    lo = (Gf - hi.astype(np.float64)).astype(np.float32).astype(np.float16)
    g16 = np.zeros((VP, 2 * K, FP), np.float16)
    g16[:, 0:K, 0:F] = np.transpose(hi, (1, 0, 2))
    g16[:, K : 2 * K, 0:F] = np.transpose(lo, (1, 0, 2))

    iota = np.arange(VOCAB, dtype=np.float32).reshape(VOCAB, 1)
    return g16, iota


_NC_CACHE = {}

# Test-harness knobs (ignored by normal kernel() use)
TRACE = False
LAST_RESULT = None


def kernel(char_ids, embed_table, conv_w, conv_b):
    global LAST_RESULT
    char_ids = np.asarray(char_ids)
    g16, iota = make_consts(
        np.asarray(embed_table), np.asarray(conv_w), np.asarray(conv_b)
    )

    if "nc" not in _NC_CACHE:
        _NC_CACHE["nc"] = build_nc()
    nc = _NC_CACHE["nc"]

    in_maps = []
    for c in range(N_CORES):
        shard = char_ids[c * NSH : (c + 1) * NSH]
        # w-major per unit, host-cast to bf16 (0..95 exact): [NUNIT, W, UNIT]
        ids_wm = np.ascontiguousarray(
            shard.reshape(NUNIT, UNIT, W).transpose(0, 2, 1)
        ).astype(mybir.dt.np(bf16))
        in_maps.append({"ids": ids_wm, "g16": g16, "iota": iota})

    kwargs = {}
    if TRACE:
        kwargs = dict(trace=True, trace_cores=list(range(N_CORES)))
    res = run_bass_kernel_spmd(nc, in_maps, core_ids=list(range(N_CORES)), **kwargs)
    LAST_RESULT = res

    out = np.empty((N, F), np.float32)
    for c in range(N_CORES):
        o = res.results[c]["out"]  # [NGROUP, F, GROUP]
        out[c * NSH : (c + 1) * NSH] = o.transpose(0, 2, 1).reshape(NSH, F)
    return out
